# revision 21
# baseline (speedup 1.0000x reference)
"""Trainium2 Bass kernel for the news-attention module (bf16 pipeline).

Computes, per batch b:
    hist = [history_repr | pos_emb[positions]]            [H, 500]
    cand = [candidate_repr | pos_emb[1]]                  [N, 500]
    hc = cand @ Wc.T ; hh = hist @ Wh.T                   [*, 200]
    a[n,h] = w2 . relu(hc[n] + hh[h] + b1)
    alpha = softmax_h(mask ? a : -1e9)
    out1 = alpha @ hist ; out2 = cand

Structure (v2):
  - all compute tensors bf16 (fp32 only for PSUM, logits softmax, outputs):
    gpsimd cast-DMAs load HBM fp32 -> SBUF bf16 directly.
  - GEMM contraction padded to 4 chunks of K=128 (f 0..511, zero-padded
    weights); inputs transposed on PE in [100,128] blocks.
  - position gather folded into matmuls: pos part of hh = onehot(pos) @ E
    with E = pos_emb @ Wh2.T; candidate pos part + b1 folded into a
    per-partition bias column c0 applied during PSUM evacuation (with the
    fp32->bf16 convert and element duplication for the DVE 2x fast path).
  - hidden built per batch-pair in [a-chunk, (b2, n, h-pad-64)] bf16 with
    zero-stride broadcast APs on the DVE; relu via tensor_scalar_max (4x).
  - w2 matvec in column form with h padded to 64: each lhsT chunk is
    exactly 128 weight columns (one n-pair x 64 h) -> FWL-eligible loads,
    output lands dense on 128 PSUM partitions as (n-parity, h).
  - logits keep fp32 through transpose/softmax for accuracy; softmax is
    batched across all batches in [25c, b, parity, h] layout.
  - final attention: per batch one PE transpose of probs -> [(parity,h64),
    25] bf16, then 2 matmuls against a partition-duplicated bf16 hist
    (rows 0-49 and 64-113) with 1/sum folded into the PSUM evacuation.
  - candidate passthrough entirely DRAM->DRAM (no SBUF round trip).

Sharding: data-parallel over batch, 8 batches per core on 8 cores.
Params replicated. Full inputs in, full outputs out.
"""

import sys

for _p in ("/opt/trn_rl_repo",):
    if _p not in sys.path:
        sys.path.insert(0, _p)

import numpy as np

import concourse.bass as bass
import concourse.bacc as bacc
import concourse.tile as tile
from concourse import mybir
from concourse import bass_utils
from concourse.masks import make_identity

DT = mybir.dt.float32
BF = mybir.dt.bfloat16
I32 = mybir.dt.int32
AF = mybir.ActivationFunctionType
ALU = mybir.AluOpType
AX = mybir.AxisListType

NCORES = 8
B = 64
BC = B // NCORES  # 8 batches per core
H = 50
N = 50
D = 400
P = 100
A = 200
F = D + P       # 500
J = 52
HP = 64         # h padded to 64 -> matvec chunks of exactly 128 columns
NC2 = N // 2    # 25 n-pair chunks per batch
KC = 128        # GEMM contraction chunk rows
NKC = 4         # chunks per 512-padded feature half

QB = 4  # batches per hidden-add instruction (4-dim free AP on hh broadcast)


def _bc(v, pos, n):
    """Insert a zero-stride (broadcast) dim of length n at position pos."""
    ap = [list(x) for x in v.ap]
    ap.insert(pos, [0, n])
    return bass.AP(tensor=v.tensor, offset=v.offset, ap=ap)


def _ap(v, offset_delta, ap_list):
    return bass.AP(tensor=v.tensor, offset=v.offset + offset_delta, ap=ap_list)


def _body(nc, hist_in, cand_in, mask_in, pos_in, pos_emb, w1t, pos_embT,
          b1, w2, ur_out, cand_out, tc):
    import contextlib

    ctx = contextlib.ExitStack()
    with ctx:
        consts = ctx.enter_context(tc.tile_pool(name="consts", bufs=1))
        ps = ctx.enter_context(tc.tile_pool(name="ps", bufs=1, space="PSUM"))
        psb = ctx.enter_context(tc.tile_pool(name="psb", bufs=1, space="PSUM"))
        psm = ctx.enter_context(tc.tile_pool(name="psm", bufs=2, space="PSUM"))
        pst = ctx.enter_context(tc.tile_pool(name="pst", bufs=1, space="PSUM"))
        pse = ctx.enter_context(tc.tile_pool(name="pse", bufs=1, space="PSUM"))
        purp = ctx.enter_context(tc.tile_pool(name="purp", bufs=2, space="PSUM"))
        amcp = ctx.enter_context(tc.tile_pool(name="amcp", bufs=2))
        eTp = ctx.enter_context(tc.tile_pool(name="eTp", bufs=2))

        # ---------------- identities ----------------
        ident = consts.tile([128, 128], DT)
        make_identity(nc, ident)
        identB = consts.tile([128, 128], BF)
        make_identity(nc, identB)

        # ---------------- bf16 cast loads (gpsimd SWDGE) ----------------
        # inputs first (transposes gate on them), then weights.
        cand_all = consts.tile([100, 4, 4 * KC], BF)   # [(hf,r), b-pair, f512]
        hist_all = consts.tile([100, 4, 4 * KC], BF)
        for hf in range(2):
            src_c = _ap(cand_in.ap(), hf * N * D,
                        [[D, 50], [2 * N * D, 4], [1, D]])
            nc.gpsimd.dma_start(out=cand_all[hf * 50:(hf + 1) * 50, :, 0:D],
                                in_=src_c)
        for hf in range(2):
            src_h = _ap(hist_in.ap(), hf * H * D,
                        [[D, 50], [2 * H * D, 4], [1, D]])
            nc.gpsimd.dma_start(out=hist_all[hf * 50:(hf + 1) * 50, :, 0:D],
                                in_=src_h)
        # zero the padded feature columns so transposed pad rows are clean
        nc.vector.memset(cand_all[:, :, D:4 * KC], 0.0)
        nc.scalar.memzero(hist_all[:, :, D:4 * KC])

        # W1T in 8 zero-padded K-chunks of 128: q0-3 cand f, q4-7 hist f
        w1T = consts.tile([KC, 8, A], BF)
        nc.vector.memset(w1T, 0.0)
        for half in range(2):
            base = half * F
            nc.gpsimd.dma_start(
                out=w1T[:, 4 * half:4 * half + 3, :],
                in_=_ap(w1t.ap(), base * A, [[A, KC], [KC * A, 3], [1, A]]))
            nc.gpsimd.dma_start(
                out=w1T[0:F - 3 * KC, 4 * half + 3, :],
                in_=_ap(w1t.ap(), (base + 3 * KC) * A,
                        [[A, F - 3 * KC], [1, A]]))

        # Wc2 (f 400:500) and Wh2 (f 900:1000) as [100, 2, A]
        wpos = consts.tile([100, 2, A], BF)
        nc.gpsimd.dma_start(
            out=wpos, in_=_ap(w1t.ap(), D * A, [[A, 100], [F * A, 2], [1, A]]))

        pos_emb_s = consts.tile([J, P], BF)
        nc.gpsimd.dma_start(out=pos_emb_s, in_=pos_emb.ap())
        posT = consts.tile([P, J], BF)
        nc.gpsimd.dma_start(out=posT, in_=pos_embT.ap())
        w2col = consts.tile([100, 2], BF)
        nc.gpsimd.dma_start(out=w2col,
                            in_=w2.ap().rearrange("(c p) -> p c", p=100))
        b1row = consts.tile([1, A], DT)
        nc.sync.dma_start(out=b1row, in_=b1.ap())
        one11 = consts.tile([1, 1], DT)
        nc.vector.memset(one11, 1.0)

        # mask bias (mask-1)*1e9 in [c, b, h] layout (broadcast over c; the
        # n-parity broadcast happens in the softmax add's AP)
        mb = consts.tile([NC2, BC, HP], BF)
        mbv = _ap(mb, 0, [list(mb.ap[0]), [HP, BC], [1, H]])
        nc.gpsimd.dma_start(
            out=mbv,
            in_=_ap(mask_in.ap(), 0, [[0, NC2], [H, BC], [1, H]]))
        nc.scalar.activation(out=mbv, in_=mbv, func=AF.Copy,
                             bias=-1e9, scale=1e9)

        # one-hot of positions, transposed: onehot[j, b*H+h] = (pos[b,h]==j)
        pos52 = consts.tile([J, BC * H], I32)
        nc.gpsimd.dma_start(out=pos52, in_=_bc(pos_in.ap(), 0, J))
        iot = consts.tile([J, BC * H], I32)
        nc.gpsimd.iota(iot, pattern=[[0, BC * H]], base=0, channel_multiplier=1)
        onehot_s = consts.tile([J, BC * H], BF)
        nc.vector.tensor_tensor(out=onehot_s, in0=iot, in1=pos52, op=ALU.is_equal)

        # candidate passthrough entirely in DRAM (independent of everything)
        nc.sync.dma_start(
            out=_ap(cand_out.ap(), 0, [[N * F, BC], [F, N], [1, D]]),
            in_=cand_in.ap())

        # hist with position columns, bf16, duplicated at partitions 0 and 64
        # (needed only by the final attention matmuls -> last in queue)
        histf2 = consts.tile([KC, BC, F], BF)
        for nr in range(2):
            nc.gpsimd.dma_start(
                out=histf2[64 * nr:64 * nr + H, :, 0:D],
                in_=_ap(hist_in.ap(), 0, [[D, H], [H * D, BC], [1, D]]))
        nc.gpsimd.dma_start(
            out=cand_out.ap()[:, :, D:F],
            in_=_bc(_bc(pos_emb.ap()[1:2, :], 0, N), 0, BC))

        # E[j, a] = pos_emb @ Wh2.T
        E_s = consts.tile([J, A], BF)
        psE = ps.tile([J, A], DT, tag="ps")
        nc.tensor.matmul(psE, lhsT=posT[:, :], rhs=wpos[:, 1, :],
                         start=True, stop=True)
        nc.vector.tensor_copy(out=E_s, in_=psE)

        # c0[a] = Wc2 @ pos_emb[1] + b1 as two per-partition bias columns
        c0col = consts.tile([100, 2], DT)
        for ac in range(2):
            asl = slice(ac * 100, (ac + 1) * 100)
            psc = ps.tile([100, 1], DT, tag="ps")
            nc.tensor.matmul(psc, lhsT=wpos[:, 0, asl], rhs=posT[:, 1:2],
                             start=True, stop=False)
            nc.tensor.matmul(psc, lhsT=b1row[:, asl], rhs=one11[:, :],
                             start=False, stop=True)
            nc.scalar.copy(out=c0col[:, ac:ac + 1], in_=psc)

        # ---------------- input transposes (PE, 128-col chunks) ----------
        candT = consts.tile([KC, 4, BC * N], BF)  # [f-chunk, k, (b,n)]
        histT = consts.tile([KC, 4, BC * H], BF)
        for k in range(4):
            ptc = psb.tile([KC, 4, 100], BF, tag="psb")
            for g in range(4):
                nc.tensor.transpose(
                    ptc[:, g, :],
                    cand_all[:, g, k * KC:(k + 1) * KC],
                    identB[:100, :100])
            nc.vector.tensor_copy(out=candT[:, k, :], in_=ptc)
            pth = psb.tile([KC, 4, 100], BF, tag="psb")
            for g in range(4):
                nc.tensor.transpose(
                    pth[:, g, :],
                    hist_all[:, g, k * KC:(k + 1) * KC],
                    identB[:100, :100])
            nc.scalar.copy(out=histT[:, k, :], in_=pth)

        # ------- GEMMs: hcT2[a, (b,n), dup2] (duplicated), hhT[a, (b,h)] --
        hcT2 = consts.tile([100, 2, BC * N, 2], BF)
        hhT = consts.tile([100, 2, BC * H], BF)
        for ac in range(2):
            asl = slice(ac * 100, (ac + 1) * 100)
            pg = ps.tile([100, BC * N], DT, tag="ps")
            for k in range(4):
                nc.tensor.matmul(pg, lhsT=w1T[:, k, asl],
                                 rhs=candT[:, k, :],
                                 start=(k == 0), stop=(k == 3))
            # evacuate + c0 bias, duplicating each element (dup2 dim)
            nc.scalar.activation(out=hcT2[:, ac, :, :], in_=_bc(pg[:, :], 2, 2),
                                 func=AF.Identity, bias=c0col[:, ac:ac + 1],
                                 scale=1.0)

            ph = ps.tile([100, BC * H], DT, tag="ps")
            for k in range(4):
                nc.tensor.matmul(ph, lhsT=w1T[:, 4 + k, asl],
                                 rhs=histT[:, k, :],
                                 start=(k == 0), stop=False)
            nc.tensor.matmul(ph, lhsT=E_s[:, asl], rhs=onehot_s[:, :],
                             start=False, stop=True)
            nc.vector.tensor_copy(out=hhT[:, ac, :], in_=ph)

        # position part of histf2 via one-hot gather matmuls (4 batches per
        # tile; each batch computed twice, at partitions 0 and 64)
        for quad in range(BC // 4):
            ppg = ps.tile([KC, 4, P], DT, tag="ps")
            for i in range(4):
                b = quad * 4 + i
                oh = onehot_s[:, b * H:(b + 1) * H]
                nc.tensor.matmul(ppg[0:H, i, :], lhsT=oh,
                                 rhs=pos_emb_s[:, :], start=True, stop=True)
                nc.tensor.matmul(ppg[64:64 + H, i, :], lhsT=oh,
                                 rhs=pos_emb_s[:, :], start=True, stop=True)
            for nr in range(2):
                nc.scalar.copy(
                    out=histf2[64 * nr:64 * nr + H, 4 * quad:4 * quad + 4, D:F],
                    in_=ppg[64 * nr:64 * nr + H, :, :])

        # ---------------- hidden + relu (bf16, h padded to 64) -----------
        # one fixed tile per (quad, ac): no rotation, pads zeroed exactly once
        nquad = BC // QB
        hids = [[consts.tile([100, QB, N, HP], BF, name=f"hid{q}_{ac}")
                 for ac in range(2)] for q in range(nquad)]
        for q in range(nquad):
            for ac in range(2):
                t = hids[q][ac]
                pad = _ap(t, H, [list(t.ap[0]), [HP, QB * N], [1, HP - H]])
                if (q + ac) % 2 == 0:
                    nc.vector.memset(pad, 0.0)
                else:
                    nc.scalar.memzero(pad)

        amr = consts.tile([NC2, BC, 2, HP], DT)
        ex = consts.tile([NC2, BC, 2, HP], BF)
        nc.gpsimd.memset(
            _ap(ex, H, [list(ex.ap[0]), [HP, BC * 2], [1, HP - H]]), 0.0)

        for q in range(nquad):
            for ac in range(2):
                hid = hids[q][ac]
                pstp = list(hid.ap[0])
                # out: [p, (b*n fused), hq25, hr2], pad columns skipped
                out_v = _ap(hid, 0, [pstp, [HP, QB * N], [2, H // 2], [1, 2]])
                v = hcT2[:, ac, q * QB * N:(q + 1) * QB * N, :]
                hcb = _bc(v, 2, H // 2)          # [p, QB*N, 25, 2]
                w = hhT[:, ac, q * QB * H:(q + 1) * QB * H]
                hhb = _ap(w, 0, [list(w.ap[0]), [H, QB], [0, N],
                                 [2, H // 2], [1, 2]])
                nc.vector.tensor_add(out=out_v, in0=hcb, in1=hhb)
                nc.vector.tensor_scalar_max(out=out_v, in0=out_v, scalar1=0.0)

            # ---- w2 matvec (column form, 128-col FWL chunks) per batch ----
            for i in range(QB):
                b = q * QB + i
                amc = psm.tile([KC, NC2], DT, tag="amc")
                for c in range(NC2):
                    for ac in range(2):
                        lhs = hids[q][ac][:, i, 2 * c:2 * c + 2, :]
                        nc.tensor.matmul(amc[:, c:c + 1], lhsT=lhs,
                                         rhs=w2col[:, ac:ac + 1],
                                         start=(ac == 0), stop=(ac == 1))
                amcs = amcp.tile([KC, NC2], DT, tag="amcs")
                nc.scalar.copy(out=amcs, in_=amc)
                amT = pst.tile([NC2, KC], DT, tag="amT")
                nc.tensor.transpose(amT[:, :], amcs[:, :], ident)
                nc.scalar.copy(out=amr[:, b, :, :], in_=amT)

        # ---------------- batched mask + softmax over h -------------------
        nm = consts.tile([NC2, BC, 2], DT)
        am2 = consts.tile([NC2, BC, 2, HP], DT)
        rs = consts.tile([NC2, BC, 2], DT)
        for half in range(2):
            bs = slice(half * (BC // 2), (half + 1) * (BC // 2))
            nb = BC // 2
            off = half * nb * 2 * HP
            amm_v = _ap(amr, off, [list(amr.ap[0]), [2 * HP, nb], [HP, 2], [1, H]])
            mb_v = _ap(mb, half * nb * HP,
                       [list(mb.ap[0]), [HP, nb], [0, 2], [1, H]])
            nc.vector.tensor_add(out=amm_v, in0=amm_v, in1=mb_v)
            nm_v = _ap(nm, half * nb * 2, [list(nm.ap[0]), [2, nb], [1, 2]])
            nc.vector.tensor_reduce(out=nm_v, in_=amm_v, axis=AX.X, op=ALU.max,
                                    negate=True)
            am2_v = _ap(am2, off, [list(am2.ap[0]), [2 * HP, nb], [HP, 2], [1, H]])
            nc.vector.tensor_add(out=am2_v, in0=amm_v,
                                 in1=_bc(nm_v, 3, H))
            ex_v = _ap(ex, off, [list(ex.ap[0]), [2 * HP, nb], [HP, 2], [1, H]])
            nc.scalar.activation(out=ex_v, in_=am2_v, func=AF.Exp)
            ssum_v = _ap(rs, half * nb * 2, [list(rs.ap[0]), [2, nb], [1, 2]])
            nc.vector.tensor_reduce(out=ssum_v, in_=ex_v, axis=AX.X, op=ALU.add)
        nc.vector.reciprocal(rs, rs)

        # ---------------- attention-weighted history ----------------------
        urs = consts.tile([NC2, 2, BC, F], DT)
        for b in range(BC):
            peT = pse.tile([KC, NC2], BF, tag="peT")
            nc.tensor.transpose(peT[:, :], ex[:, b, :, :],
                                identB[:NC2, :NC2])
            eTs = eTp.tile([KC, NC2], BF, tag="eTs")
            nc.vector.tensor_copy(out=eTs, in_=peT)
            for nr in range(2):
                pur = purp.tile([NC2, F], DT, tag="pur")
                nc.tensor.matmul(pur, lhsT=eTs[64 * nr:64 * nr + H, :],
                                 rhs=histf2[64 * nr:64 * nr + H, b, :],
                                 start=True, stop=True)
                if nr == 0:
                    nc.scalar.activation(out=urs[:, nr, b, :], in_=pur,
                                         func=AF.Copy, scale=rs[:, b, nr:nr + 1])
                else:
                    nc.vector.tensor_scalar_mul(
                        out=urs[:, nr, b, :], in0=pur,
                        scalar1=rs[:, b, nr:nr + 1])
            eng = nc.sync if b % 2 == 0 else nc.scalar
            eng.dma_start(
                out=_ap(ur_out.ap(), b * N * F, [[2 * F, NC2], [F, 2], [1, F]]),
                in_=urs[:, :, b, :])


def build(debug=False, reps=1):
    nc = bacc.Bacc("TRN2", target_bir_lowering=False, debug=debug)
    hist_in = nc.dram_tensor("hist_in", [BC, H, D], DT, kind="ExternalInput")
    cand_in = nc.dram_tensor("cand_in", [BC, N, D], DT, kind="ExternalInput")
    mask_in = nc.dram_tensor("mask_in", [BC, H], DT, kind="ExternalInput")
    pos_in = nc.dram_tensor("pos_in", [BC, H], I32, kind="ExternalInput")
    pos_emb = nc.dram_tensor("pos_emb", [J, P], DT, kind="ExternalInput")
    w1t = nc.dram_tensor("w1t", [2 * F, A], DT, kind="ExternalInput")
    pos_embT = nc.dram_tensor("pos_embT", [P, J], DT, kind="ExternalInput")
    b1 = nc.dram_tensor("b1", [A], DT, kind="ExternalInput")
    w2 = nc.dram_tensor("w2", [A], DT, kind="ExternalInput")
    ur_out = nc.dram_tensor("ur_out", [BC, N, F], DT, kind="ExternalOutput")
    cand_out = nc.dram_tensor("cand_out", [BC, N, F], DT, kind="ExternalOutput")

    with tile.TileContext(nc) as tc:
        for _ in range(reps):
            _body(nc, hist_in, cand_in, mask_in, pos_in, pos_emb, w1t,
                  pos_embT, b1, w2, ur_out, cand_out, tc)
    nc.compile()
    return nc


_NC = None


def _get_nc():
    global _NC
    if _NC is None:
        _NC = build(debug=False)
    return _NC


def make_in_maps(history_repr, candidate_repr, user_history_mask,
                 user_history_position, pos_emb, W1, b1, w2):
    hist = np.ascontiguousarray(np.asarray(history_repr, np.float32))
    cand = np.ascontiguousarray(np.asarray(candidate_repr, np.float32))
    mask = np.asarray(user_history_mask).astype(np.float32)
    pos = np.asarray(user_history_position).astype(np.int32)
    pe = np.ascontiguousarray(np.asarray(pos_emb, np.float32))
    w1t = np.ascontiguousarray(np.asarray(W1, np.float32).T)
    peT = np.ascontiguousarray(pe.T)
    b1_ = np.ascontiguousarray(np.asarray(b1, np.float32))
    w2_ = np.ascontiguousarray(np.asarray(w2, np.float32))
    in_maps = []
    for c in range(NCORES):
        sl = slice(c * BC, (c + 1) * BC)
        in_maps.append({
            "hist_in": hist[sl], "cand_in": cand[sl],
            "mask_in": mask[sl], "pos_in": pos[sl],
            "pos_emb": pe, "w1t": w1t, "pos_embT": peT,
            "b1": b1_, "w2": w2_,
        })
    return in_maps


def kernel(history_repr, candidate_repr, user_history_mask,
           user_history_position, pos_emb, W1, b1, w2, b2=None, **_ignored):
    # b2 shifts every logit equally -> cancels in softmax; unused.
    nc = _get_nc()
    in_maps = make_in_maps(history_repr, candidate_repr, user_history_mask,
                           user_history_position, pos_emb, W1, b1, w2)
    res = bass_utils.run_bass_kernel_spmd(nc, in_maps, list(range(NCORES)))
    ur = np.concatenate([res.results[c]["ur_out"] for c in range(NCORES)], 0)
    cand = np.concatenate([res.results[c]["cand_out"] for c in range(NCORES)], 0)
    return ur, cand
